# revision 62
# baseline (speedup 1.0000x reference)
"""Multi-head attention + output projection (nn_AttentionBase) on 8 Trainium2
NeuronCores.

Reference computation (B=2, S=2048, E=2048, H=16, c=128, fp32):
    scores  = einsum('bqhc,bkhc->bhqk', q/sqrt(c), k)
    weights = softmax(scores + mask_bias_on_keys)
    out     = einsum('bhqk,bkhc->bqhc', weights, v) @ w_out.T

Sharding: 8 cores = (batch b: 2) x (query block of 512: 4). Each core computes
all 16 heads for its 512 queries against the valid keys of its batch, then
applies the full output projection for its rows. No inter-core reduction is
needed; the host concatenates the 8 [512, 2048] results.

Mask sparsity: the attention mask is a padding mask on keys (~50% masked).
Softmax+attention are permutation-invariant over keys, so the host sorts each
batch's keys (and values) valid-first. The kernel then only processes the
first NCHE = ceil(max_valid/128) key chunks; fully-masked tail chunks
contribute exactly zero (exp(-30000) == 0 in fp32) and are skipped. Chunks
below NFULL = min_b(valid_b)//128 are valid for BOTH batches, so their exp
runs with a constant zero bias and can be batched across PSUM banks; only the
few boundary chunks need the per-partition mask bias.

Per-core dataflow (matmuls bf16 with fp32 PSUM accumulation):
  scoresT[sk,sq] = kT.T @ qT            (per 128-key chunk, PE)
  pT = exp(scoresT * c^-0.5 [+ maskb])  (ScalarE; zero-bias chunks batched two
                                         per ACT over a 2-bank PSUM group to
                                         amortize the ~352-cycle ACT overhead)
  attnT[c,sq]   += v_chunk.T @ pT       (PE, accumulated over key chunks)
  s_pt[sk%128,sq] = sum_j pT_j          (GpSimd tensor adds - off the PE)
  l[1,sq]        = ones.T @ s_pt        (PE, ONE M=1 matmul per head)
  attn_sb[c,sq]  = attnT * (1/l)        (VectorE; approx reciprocal + 1/l
                                         partition-broadcast via a DRAM bounce)
  y[sq,e_out]    = sum_h attn_sb_h.T @ w_outT  (PE, contraction over e_in)
"""
import sys

sys.path.insert(0, "/opt/trn_rl_repo")

import math

import ml_dtypes
import numpy as np

import concourse.bass as bass
import concourse.mybir as mybir
import concourse.tile as tile

B, S, E = 2, 2048, 2048
H, C = 16, 128
SQ = 512          # queries per core
NSQT = SQ // 128   # 4 query subtiles
NNT = E // 512     # 4 output column tiles
MASK_NEG = -30000.0
BF16 = mybir.dt.bfloat16
F32 = mybir.dt.float32
NHALF_Y = 7        # output tiles whose first-half contraction runs in phase A


_WAIT_LIMIT = 1


def _split_excess_waits(nc, limit=_WAIT_LIMIT):
    """The walrus build in this container rejects instructions carrying more
    than one semaphore wait ("Too many sync wait commands"). Move excess waits
    onto NoOps inserted just before the instruction on the same engine (engine
    streams execute in block order, so the waits still gate the instruction)."""
    for f in nc.m.functions:
        for bb in f.blocks:
            new = []
            changed = False
            for inst in bb.instructions:
                si = inst.sync_info
                if si is not None and len(si.on_wait) > limit:
                    waits = list(si.on_wait)
                    excess, keep = waits[:-limit], waits[-limit:]
                    for k in range(0, len(excess), limit):
                        nop = mybir.InstNoOp(
                            name=f"{inst.name}-wsplit{k}",
                            sync_info=mybir.SyncInfo(
                                on_wait=excess[k:k + limit], on_update=[]
                            ),
                            bass_nofuse=True,
                            engine=inst.engine,
                        )
                        new.append(nop)
                    inst.sync_info = mybir.SyncInfo(
                        on_wait=keep, on_update=list(si.on_update)
                    )
                    changed = True
                new.append(inst)
            if changed:
                bb.instructions = new


def _build_program(nfull, nche):
    """Build the per-core program for nche key chunks, of which the first
    nfull are fully valid for both batches (constant zero mask bias)."""
    nc = bass.Bass()
    nk = nche * 128
    qT = nc.declare_dram_parameter("qT", [H, C, SQ], BF16, isOutput=False)
    kT = nc.declare_dram_parameter("kT", [H, C, nk], BF16, isOutput=False)
    v = nc.declare_dram_parameter("v", [H, 128, nche, C], BF16, isOutput=False)
    wT = nc.declare_dram_parameter("wT", [E, E], BF16, isOutput=False)
    maskb = nc.declare_dram_parameter("maskb", [128, nche], F32, isOutput=False)
    y = nc.declare_dram_parameter("y", [SQ, E], F32, isOutput=True)

    scale = 1.0 / math.sqrt(C)

    # ACT groups: (chunk0, nchunks, zero_bias)
    groups = []
    for g in range(nfull // 2):
        groups.append((2 * g, 2, True))
    if nfull % 2:
        groups.append((nfull - 1, 1, True))
    for j in range(nfull, nche):
        groups.append((j, 1, False))

    with tile.TileContext(nc) as tc:
        with (
            tc.tile_pool(name="consts", bufs=1) as consts,
            tc.tile_pool(name="wpool", bufs=1) as wpool,
            tc.tile_pool(name="attn_all", bufs=1) as attn_all,
            tc.tile_pool(name="kv", bufs=3) as kv,
            tc.tile_pool(name="pt", bufs=10) as ptpool,
            tc.tile_pool(name="spt", bufs=3) as sptpool,
            tc.tile_pool(name="small", bufs=3) as small,
            tc.tile_pool(name="lbc", bufs=6) as lbc,
            tc.tile_pool(name="ldram", bufs=3, space="DRAM") as ldram,
            tc.tile_pool(name="yout", bufs=3) as yout,
            tc.tile_pool(name="ypart", bufs=1) as ypart,
            tc.tile_pool(name="psS", bufs=2, space="PSUM") as psS,
            tc.tile_pool(name="psA", bufs=3, space="PSUM") as psA,
            tc.tile_pool(name="psL", bufs=1, space="PSUM") as psL,
        ):
            ones = consts.tile([128, 1], BF16)
            nc.vector.memset(ones, 1.0)
            maskb_sb = consts.tile([128, nche], F32)
            nc.sync.dma_start(maskb_sb, maskb[:, :])
            # dummy exp at t=0: triggers the ~2.7us ACT_TABLE_LOAD while the
            # first K/Q DMAs are still in flight instead of on the first
            # real exp
            warm = consts.tile([1, 1], F32, tag="warm")
            nc.scalar.activation(warm, ones[0:1, 0:1],
                                 mybir.ActivationFunctionType.Exp)


            w_sb = wpool.tile([128, E // 128, E], BF16)
            attn_tiles = [attn_all.tile([128, SQ], BF16, tag=f"a{h}",
                                        name=f"attn{h}") for h in range(H)]

            pending = []
            yraw_tiles = {}
            for h in range(H):
                # K^T in one DMA per head: SP's ~650ns per-dispatch cost
                # outweighs finer-grained load gating — EXCEPT for the first
                # two heads, where the 288KB load on a single ~50GB/s queue
                # (~5.7us) gates time-to-first-matmul; split those across 4
                # queues (DMA dispatch round-robins).
                ktall = kv.tile([128, nk], BF16, tag="kt")
                qt = kv.tile([128, SQ], BF16, tag="qt")
                if h < 2:
                    step = (nche + 3) // 4 * 128
                    nc.sync.dma_start(ktall[:, :step], kT[h][:, :step])
                    nc.sync.dma_start(qt, qT[h])
                    for o in range(step, nk, step):
                        e = min(o + step, nk)
                        nc.sync.dma_start(ktall[:, o:e], kT[h][:, o:e])
                else:
                    nc.sync.dma_start(ktall, kT[h])
                    nc.sync.dma_start(qt, qT[h])
                vt = kv.tile([128, nche, C], BF16, tag="vt")
                nc.sync.dma_start(vt, v[h])
                # head h's slice of the projection weights, used in phase B
                nc.gpsimd.dma_start(w_sb[:, h, :], wT[h * 128:(h + 1) * 128, :])

                ps_at = psA.tile([128, SQ], F32, tag="at")

                # scores + exp per group; zero-bias groups batch 2 chunks
                # into one ACT over a 2-bank PSUM tile.
                pt_slices = []  # chunk j -> AP of its [128, SQ] exp tile
                for gi, (c0, n, zb) in enumerate(groups):
                    ps_g = psS.tile([128, n * SQ], F32)
                    for jj in range(n):
                        nc.tensor.matmul(
                            ps_g[:, jj * SQ:(jj + 1) * SQ],
                            lhsT=ktall[:, (c0 + jj) * 128:(c0 + jj + 1) * 128],
                            rhs=qt,
                            start=True, stop=True,
                        )
                    pt_g = ptpool.tile([128, n * SQ], BF16)
                    bias = 0.0 if zb else maskb_sb[:, c0:c0 + 1]
                    nc.scalar.activation(
                        pt_g, ps_g, mybir.ActivationFunctionType.Exp,
                        bias=bias, scale=scale,
                    )
                    for jj in range(n):
                        pt_slices.append(pt_g[:, jj * SQ:(jj + 1) * SQ])

                for j in range(nche):
                    nc.tensor.matmul(
                        ps_at, lhsT=vt[:, j, :], rhs=pt_slices[j],
                        start=(j == 0), stop=(j == nche - 1),
                    )

                # softmax denominator: tree-accumulate the exp tiles in bf16
                # (DVE 2x mode; level-0 adds split with GpSimd), as TWO
                # half-trees so each l-matmul depends on a shallow, early-
                # finishing sum. bf16 rounding (~2^-9 per level) averages
                # out across the 128-partition contraction: l err ~0.04%.
                def tree(slices, who, gps_l0):
                    level = list(slices)
                    li = 0
                    while len(level) > 1:
                        nxt = []
                        for i in range(0, len(level) - 1, 2):
                            t = sptpool.tile([128, SQ], BF16,
                                             tag=f"s{who}{li}{i}")
                            eng = (nc.gpsimd if (li == 0 and i < gps_l0)
                                   else nc.vector)
                            eng.tensor_add(t, level[i], level[i + 1])
                            nxt.append(t)
                        if len(level) % 2:
                            nxt.append(level[-1])
                        level = nxt
                        li += 1
                    return level[0]
                half = min(4, nche - 1) if nche > 1 else 1
                s_halves = [tree(pt_slices[:half], "L", 4)]
                if nche > half:
                    s_halves.append(tree(pt_slices[half:], "R", 2))

                def finish_head(h=h, ps_at=ps_at, s_halves=s_halves):
                    # Deferred by one head so the M=1 l-matmuls never block
                    # PE's in-order stream waiting on the DVE/GpSimd tree.
                    ps_l = psL.tile([32, SQ], F32, tag="ly")
                    for si, s in enumerate(s_halves):
                        nc.tensor.matmul(ps_l[0:1, :], lhsT=ones, rhs=s,
                                         start=(si == 0),
                                         stop=(si == len(s_halves) - 1))
                    # 1/l: DVE reciprocal on [1,512] is ~3.3us (512 elems on
                    # one lane). DVE 32x32 block transposes spread l across
                    # 32 lanes: t1[i,32k] = l[32k+i]; reciprocal on the
                    # stride-32 view (16 elems/lane); transpose back so t2
                    # row 0 is 1/l in q order; DRAM-bounce broadcast. The
                    # bounce DMAs ride the GpSimd queue to keep SP's in-order
                    # dispatch free of data-dependent waits.
                    t1 = small.tile([32, SQ], F32, tag="t1")
                    nc.vector.transpose(t1, ps_l)
                    rt = small.tile([32, SQ], F32, tag="rt")
                    nc.vector.reciprocal(rt[:, ::32], t1[:, ::32])
                    t2 = small.tile([32, SQ], F32, tag="t2")
                    nc.vector.transpose(t2, rt)
                    ld = ldram.tile([1, SQ], F32)
                    nc.sync.dma_start(ld, t2[0:1, :])
                    lb = lbc.tile([128, SQ], F32)
                    nc.sync.dma_start(
                        lb,
                        bass.AP(tensor=ld.tensor, offset=ld.offset,
                                ap=[[0, 128]] + list(ld.ap[1:])),
                    )
                    # normalize straight out of PSUM
                    nc.vector.tensor_mul(attn_tiles[h], ps_at, lb)

                pending.append(finish_head)
                if len(pending) > 1:
                    pending.pop(0)()

                # Late in phase A the PE has ACT-gated bubbles; fill them by
                # pre-computing the first-8-head contraction of one output
                # tile per head (no ACT/DVE dependency). Partial sums park in
                # SBUF f32; phase B finishes heads 8-15 and adds them back.
                if h >= H - NHALF_Y:
                    g = h - (H - NHALF_Y)
                    i, n = divmod(g, NNT)
                    # contract the first 8 heads (all safely normalized by
                    # head 9); more than 8 overflows the ACT-gated PE bubble
                    ecn = H // 2
                    ps_hy = psL.tile([128, 512], F32, tag="ly")
                    for ec in range(ecn):
                        nc.tensor.matmul(
                            ps_hy,
                            lhsT=attn_tiles[ec][:, i * 128:(i + 1) * 128],
                            rhs=w_sb[:, ec, n * 512:(n + 1) * 512],
                            start=(ec == 0), stop=(ec == ecn - 1),
                        )
                    yr = ypart.tile([128, 512], F32, tag=f"yr{g}")
                    nc.vector.tensor_copy(yr, ps_hy)
                    yraw_tiles[g] = (yr, ecn)
            for p in pending:
                p()

            # full groups first, prefilled groups (8 MMs + DVE add) last, so
            # the final y stores chase a shorter dependency chain
            order = [g for g in range(NSQT * NNT) if g not in yraw_tiles] + \
                    [g for g in range(NSQT * NNT) if g in yraw_tiles]
            for gi_b, g in enumerate(order):
                if True:
                    i, n = divmod(g, NNT)
                    # alternate accumulator banks between the two pools so
                    # consecutive groups double-buffer within 8 banks
                    pool = psA if gi_b % 2 == 0 else psL
                    tag = "at" if pool is psA else "ly"
                    ps_y = pool.tile([128, 512], F32, tag=tag)
                    ec0 = yraw_tiles[g][1] if g in yraw_tiles else 0
                    for ec in range(ec0, H):
                        nc.tensor.matmul(
                            ps_y,
                            lhsT=attn_tiles[ec][:, i * 128:(i + 1) * 128],
                            rhs=w_sb[:, ec, n * 512:(n + 1) * 512],
                            start=(ec == ec0), stop=(ec == H - 1),
                        )
                    yt = yout.tile([128, 512], F32)
                    if gi_b >= NSQT * NNT - 2:
                        # last two groups: evacuate + store in halves so the
                        # final DMAs start before the whole tile is copied
                        for hf in range(2):
                            sl = slice(hf * 256, (hf + 1) * 256)
                            if g in yraw_tiles:
                                nc.vector.tensor_add(
                                    yt[:, sl], ps_y[:, sl],
                                    yraw_tiles[g][0][:, sl])
                            else:
                                nc.scalar.copy(yt[:, sl], ps_y[:, sl])
                            nc.sync.dma_start(
                                y[i * 128:(i + 1) * 128,
                                  n * 512 + hf * 256:n * 512 + (hf + 1) * 256],
                                yt[:, sl],
                            )
                    else:
                        if g in yraw_tiles:
                            nc.vector.tensor_add(yt, ps_y, yraw_tiles[g][0])
                        else:
                            nc.scalar.copy(yt, ps_y)
                        nc.sync.dma_start(
                            y[i * 128:(i + 1) * 128, n * 512:(n + 1) * 512],
                            yt,
                        )

    _split_excess_waits(nc)
    return nc


_PROGRAMS = {}


def _get_program(nfull, nche):
    key = (nfull, nche)
    if key not in _PROGRAMS:
        _PROGRAMS[key] = _build_program(nfull, nche)
    return _PROGRAMS[key]


def _make_in_maps(keys, values, queries, attention_mask, w_out):
    bf = ml_dtypes.bfloat16
    wT_host = np.ascontiguousarray(w_out.astype(bf).T)

    nv = attention_mask.sum(axis=1).astype(np.int64)  # valid keys per batch
    nfull = int(nv.min()) // 128
    nche = max(1, int(-(-int(nv.max()) // 128)))
    nk = nche * 128

    per_batch = []
    for b in range(B):
        order = np.argsort(~attention_mask[b], kind="stable")[:nk]
        kb = keys[b][order].astype(bf).reshape(nk, H, C)
        kT_host = np.ascontiguousarray(kb.transpose(1, 2, 0))
        vb = values[b][order].astype(bf).reshape(nche, 128, H, C)
        v_host = np.ascontiguousarray(vb.transpose(2, 1, 0, 3))
        mb = np.where(attention_mask[b][order], 0.0, MASK_NEG).astype(np.float32)
        maskb_host = np.ascontiguousarray(mb.reshape(nche, 128).T)
        per_batch.append((kT_host, v_host, maskb_host))

    in_maps = []
    for core in range(8):
        b = core // 4
        q0 = (core % 4) * SQ
        qb = queries[b, q0:q0 + SQ].astype(bf).reshape(SQ, H, C)
        qT_host = np.ascontiguousarray(qb.transpose(1, 2, 0))
        kT_host, v_host, maskb_host = per_batch[b]
        in_maps.append({
            "qT": qT_host,
            "kT": kT_host,
            "v": v_host,
            "wT": wT_host,
            "maskb": maskb_host,
        })
    return in_maps, nfull, nche


def _run(inputs, trace=False, trace_cores=None):
    from concourse.bass_utils import run_bass_kernel_spmd

    in_maps, nfull, nche = _make_in_maps(**inputs)
    nc = _get_program(nfull, nche)
    res = run_bass_kernel_spmd(
        nc, in_maps, core_ids=list(range(8)),
        trace=trace, trace_cores=trace_cores,
    )
    out = np.empty((B, S, E), dtype=np.float32)
    for core in range(8):
        b = core // 4
        q0 = (core % 4) * SQ
        out[b, q0:q0 + SQ, :] = res.results[core]["y"]
    return out, res


def kernel(keys, values, queries, attention_mask, w_out):
    out, _ = _run(dict(
        keys=np.asarray(keys), values=np.asarray(values),
        queries=np.asarray(queries),
        attention_mask=np.asarray(attention_mask),
        w_out=np.asarray(w_out),
    ))
    return out
